# revision 1
# baseline (speedup 1.0000x reference)
"""Cross-attention Trainium2 kernel (Bass/Tile), SPMD over 8 NeuronCores.

Reference computation (per batch element b):
    xs = x[b].reshape(C, H*W).T            # (N, C)   N=4096 tokens
    q  = xs @ Wq + bq                      # (N, C)
    k  = ctx[b] @ Wk + bk                  # (T, C)   T=77
    v  = ctx[b] @ Wv + bv                  # (T, C)
    per head i (d=80): s_i = q_i k_i^T * d^-0.5 ; a_i = softmax(s_i)
    out = concat_i(a_i v_i) @ Wo + bo      # (N, C)
    y[b] = out.T.reshape(C, H, W)

Sharding: data-parallel over batch, 2 images per core.

On-chip layout is "transposed" (channels on partitions, tokens on the free
axis) so x needs no transpose at all:
    qT = Wq^T @ x[b]         (C, N) as 5 (128,512)-tiles per 512-token block
    sT_i = K_i qT_i          (T, 512) per head, via zero-padded (128,77)
                             lhsT pieces so rhs stays the native q tiles
                             (head rows cross 128-partition boundaries and
                             engines can only address partition 0/32/64/96)
    E = exp(sT)              ACT, softmax scale folded into K
    oT_i = [V_i |0| 1] ^T E  (97, 512); row 96 = softmax denominator r
    rinv_i = approx_recip(r) from PSUM; DMA-gathered into rall (8,512)
    R_t = S_t^T rall         (128,512) row p = rinv[head(p)] via a 0/1
                             selection matrix (PE broadcast matmul)
    outT tiles (128,512) assembled from oT head rows via DMA (DMA has no
                             partition-alignment restriction), then
    outn_t = outT_t * R_t    normalized (DVE, SBUF x PSUM)
    y = Wo^T @ outn + bo     (C, N) -> directly the output layout
All big matmuls run as float32r (full-rate fp32 on the PE at free >= 256);
every SBUF operand of an fp32r matmul must be produced with f32r dtype
(walrus "rounded to FP32r" rule), hence the convert-copies and DMA bitcasts.
"""

import numpy as np

# ---- problem constants (hardcoded per contest contract) ----
B, C, HH, WW = 16, 640, 64, 64
NTOK = HH * WW          # 4096
T = 77
CTX = 768
NH = 8
D = C // NH             # 80
SCALE = float(D) ** -0.5
N_CORES = 8
BPC = B // N_CORES      # 2 images per core
BLK = 512
NBLK = NTOK // BLK      # 8
KC = C // 128           # 5
KX = CTX // 128         # 6
RROW = 96               # denominator row in the attnV psum (legal start)

# head i covers channel rows [80i, 80i+80); split at 128-tile edges:
# (head, tile, lo, hi, dlo): rows [lo,hi) of tile `tile` = head dims
# [dlo, dlo+hi-lo)
PIECES = []
for _i in range(NH):
    _c0, _c1 = D * _i, D * (_i + 1)
    for _t in range(_c0 // 128, (_c1 - 1) // 128 + 1):
        _lo, _hi = max(_c0, 128 * _t), min(_c1, 128 * (_t + 1))
        PIECES.append((_i, _t, _lo - 128 * _t, _hi - 128 * _t, _lo - _c0))

_CACHE = {}


def _build_nc():
    from contextlib import ExitStack
    import concourse.bacc as bacc
    import concourse.tile as tile
    import concourse.mybir as mybir
    from concourse.masks import make_identity

    f32 = mybir.dt.float32
    f32r = mybir.dt.float32r
    AF = mybir.ActivationFunctionType

    nc = bacc.Bacc("TRN2", target_bir_lowering=False, debug=False,
                   num_devices=N_CORES)

    x_d = nc.dram_tensor("x", [BPC, C, NTOK], f32, kind="ExternalInput").ap()
    ctx_d = nc.dram_tensor("ctxt", [BPC, T, CTX], f32, kind="ExternalInput").ap()
    wq_d = nc.dram_tensor("wq", [C, C], f32, kind="ExternalInput").ap()
    wk_d = nc.dram_tensor("wk", [CTX, C], f32, kind="ExternalInput").ap()
    wv_d = nc.dram_tensor("wv", [CTX, C], f32, kind="ExternalInput").ap()
    wo_d = nc.dram_tensor("wo", [C, C], f32, kind="ExternalInput").ap()
    bq_d = nc.dram_tensor("bq", [C], f32, kind="ExternalInput").ap()
    bk_d = nc.dram_tensor("bk", [C], f32, kind="ExternalInput").ap()
    bv_d = nc.dram_tensor("bv", [C], f32, kind="ExternalInput").ap()
    bo_d = nc.dram_tensor("bo", [C], f32, kind="ExternalInput").ap()
    out_d = nc.dram_tensor("out", [BPC, C, NTOK], f32, kind="ExternalOutput").ap()

    with tile.TileContext(nc) as tc, ExitStack() as ctx:
        wpool = ctx.enter_context(tc.tile_pool(name="wpool", bufs=1))
        cpool = ctx.enter_context(tc.tile_pool(name="cpool", bufs=1))
        ipool = ctx.enter_context(tc.tile_pool(name="ipool", bufs=1))
        xpool = ctx.enter_context(tc.tile_pool(name="xpool", bufs=2))
        qpool = ctx.enter_context(tc.tile_pool(name="qpool", bufs=2))
        epool = ctx.enter_context(tc.tile_pool(name="epool", bufs=2))
        hpool = ctx.enter_context(tc.tile_pool(name="hpool", bufs=2))
        npool = ctx.enter_context(tc.tile_pool(name="npool", bufs=4))
        opool = ctx.enter_context(tc.tile_pool(name="opool", bufs=2))
        fpool = ctx.enter_context(tc.tile_pool(name="fpool", bufs=2))
        ppa = ctx.enter_context(tc.tile_pool(name="ppa", bufs=3, space="PSUM"))
        psr = ctx.enter_context(tc.tile_pool(name="psr", bufs=3, space="PSUM"))
        pso = ctx.enter_context(tc.tile_pool(name="pso", bufs=2, space="PSUM"))

        # ---------- persistent weights / constants ----------
        def load_w(dram, n_k, tag):
            ts = []
            for k in range(n_k):
                t = wpool.tile([128, C], f32r, tag=f"{tag}{k}")
                nc.sync.dma_start(t, dram[128 * k:128 * (k + 1), :].bitcast(f32r))
                ts.append(t)
            return ts

        wq_sb = load_w(wq_d, KC, "wq")
        wo_sb = load_w(wo_d, KC, "wo")
        wk_sb = load_w(wk_d, KX, "wk")
        wv_sb = load_w(wv_d, KX, "wv")
        bv_row = wpool.tile([1, C], f32r, tag="bvrow")
        nc.sync.dma_start(bv_row, bv_d[None, :].bitcast(f32r))

        # per-partition bias layouts: bias[128m + p] at [p, m]
        bqT = wpool.tile([128, KC], f32, tag="bqT")
        nc.sync.dma_start(bqT, bq_d.rearrange("(m p) -> p m", p=128))
        boT = wpool.tile([128, KC], f32, tag="boT")
        nc.sync.dma_start(boT, bo_d.rearrange("(m p) -> p m", p=128))
        bkT = wpool.tile([128, KC], f32, tag="bkT")
        nc.sync.dma_start(bkT, bk_d.rearrange("(m p) -> p m", p=128))
        bkTs = wpool.tile([128, KC], f32, tag="bkTs")
        nc.vector.tensor_scalar_mul(bkTs, bkT, SCALE)

        ident = cpool.tile([128, 128], f32, tag="ident")
        make_identity(nc, ident)
        zeros32 = cpool.tile([128, 128], f32, tag="zeros32")
        nc.vector.memset(zeros32, 0.0)
        ones32 = cpool.tile([128, 1], f32, tag="ones32")
        nc.vector.memset(ones32, 1.0)
        ones_row32 = cpool.tile([1, 128], f32, tag="onesrow32")
        nc.vector.memset(ones_row32, 1.0)
        ones77 = cpool.tile([1, T], f32r, tag="ones77")
        nc.vector.tensor_copy(ones77, ones_row32[:, 0:T])

        # selection matrices S_t (8,128): S_t[i,p] = 1 iff head(128t+p) == i
        S_sel = []
        for t in range(KC):
            s32 = cpool.tile([NH, 128], f32, tag=f"s32_{t}")
            nc.gpsimd.memset(s32, 1.0)
            # cond A: p + 128t - 80i >= 0
            nc.gpsimd.affine_select(
                out=s32, in_=s32, compare_op=mybir.AluOpType.is_ge, fill=0.0,
                base=128 * t, pattern=[[1, 128]], channel_multiplier=-D)
            # cond B: -p - 128t + 80i + 79 >= 0
            nc.gpsimd.affine_select(
                out=s32, in_=s32, compare_op=mybir.AluOpType.is_ge, fill=0.0,
                base=D - 1 - 128 * t, pattern=[[-1, 128]], channel_multiplier=D)
            st = cpool.tile([NH, 128], f32r, tag=f"ssel_{t}")
            nc.vector.tensor_copy(st, s32)
            S_sel.append(st)

        for b in range(BPC):
            # ---------- per-image prep ----------
            ctx_sb = ipool.tile([T, CTX], f32, tag="ctx")
            nc.sync.dma_start(ctx_sb, ctx_d[b])

            # ctx^T tiles (128, 77) x6 via PE transpose
            ctxT = []
            for k in range(KX):
                pt = psr.tile([128, T], f32, tag="psr")
                nc.tensor.transpose(pt, ctx_sb[:, 128 * k:128 * (k + 1)],
                                    ident[0:T, 0:T])
                t = ipool.tile([128, T], f32r, tag=f"ctxT{k}")
                nc.vector.tensor_copy(t, pt)
                ctxT.append(t)

            # K^T = scale * (Wk^T @ ctx^T + bk)
            ktmp = []
            for m in range(KC):
                pt = ppa.tile([128, T], f32, tag="ppa")
                for k in range(KX):
                    # N=77 is odd; fp32r needs an even moving dim -> plain f32
                    nc.tensor.matmul(
                        pt, wk_sb[k][:, 128 * m:128 * (m + 1)].bitcast(f32),
                        ctxT[k].bitcast(f32),
                        start=(k == 0), stop=(k == KX - 1))
                t = ipool.tile([128, T], f32r, tag=f"ktmp{m}")
                nc.scalar.activation(t, pt, AF.Identity,
                                     bias=bkTs[:, m:m + 1], scale=SCALE)
                ktmp.append(t)

            # zero-padded per-(head, tile) lhsT pieces for the scores matmul
            kTp = {}
            for (i, tt, lo, hi, dlo) in PIECES:
                t = ipool.tile([128, T], f32r, tag=f"kTp{i}_{tt}")
                nc.vector.tensor_copy(t, zeros32[:, 0:T])
                nc.sync.dma_start(t[lo:hi, :], ktmp[tt][lo:hi, :])
                kTp[(i, tt)] = t

            # V heads: vA_i = [V_i | zeros | ones]  (77, 97), ones col at 96
            vA = []
            for i in range(NH):
                t = ipool.tile([T, RROW + 1], f32r, tag=f"vA{i}")
                nc.vector.tensor_copy(t[:, D:RROW], zeros32[0:T, 0:RROW - D])
                nc.vector.tensor_copy(t[:, RROW:RROW + 1], ones32[0:T, :])
                vA.append(t)
            for h2 in range(2):
                pt = pso.tile([T, 320], f32, tag="pso")
                for k in range(KX):
                    nc.tensor.matmul(
                        pt, ctxT[k], wv_sb[k][:, 320 * h2:320 * (h2 + 1)],
                        start=(k == 0), stop=False)
                nc.tensor.matmul(
                    pt, ones77, bv_row[:, 320 * h2:320 * (h2 + 1)],
                    start=False, stop=True)
                for i in range(4 * h2, 4 * h2 + 4):
                    off = D * i - 320 * h2
                    nc.vector.tensor_copy(vA[i][:, 0:D], pt[:, off:off + D])

            # ---------- 512-token blocks ----------
            for n in range(NBLK):
                c0 = BLK * n
                xk = []
                for k in range(KC):
                    t = xpool.tile([128, BLK], f32r, tag=f"x{k}")
                    nc.sync.dma_start(
                        t, x_d[b, 128 * k:128 * (k + 1), c0:c0 + BLK].bitcast(f32r))
                    xk.append(t)

                # qT = Wq^T @ x (+bq), 5 m-tiles of (128, 512)
                qtmp = []
                for m in range(KC):
                    pt = ppa.tile([128, BLK], f32, tag="ppa")
                    for k in range(KC):
                        nc.tensor.matmul(
                            pt, wq_sb[k][:, 128 * m:128 * (m + 1)], xk[k],
                            start=(k == 0), stop=(k == KC - 1))
                    t = qpool.tile([128, BLK], f32r, tag=f"q{m}")
                    nc.scalar.activation(t, pt, AF.Identity,
                                         bias=bqT[:, m:m + 1])
                    qtmp.append(t)

                # per-head attention
                r8 = hpool.tile([NH, BLK], f32, tag="r8")
                onh = []
                for i in range(NH):
                    spans = [p for p in PIECES if p[0] == i]
                    sps = psr.tile([T, BLK], f32, tag="psr")
                    for j, (_, tt, _, _, _) in enumerate(spans):
                        nc.tensor.matmul(
                            sps, kTp[(i, tt)], qtmp[tt],
                            start=(j == 0), stop=(j == len(spans) - 1))
                    e_sb = epool.tile([T, BLK], f32r, tag="e")
                    nc.scalar.activation(e_sb, sps, AF.Exp)
                    opt = pso.tile([RROW + 1, BLK], f32, tag="pso")
                    nc.tensor.matmul(opt, vA[i], e_sb, start=True, stop=True)
                    rg = hpool.tile([1, BLK], f32, tag="rg")
                    t = npool.tile([D, BLK], f32, tag="onh")
                    if i % 2 == 0:
                        nc.scalar.activation(rg, opt[RROW:RROW + 1, :], AF.Copy)
                        nc.vector.tensor_copy(t, opt[0:D, :])
                    else:
                        nc.vector.tensor_copy(rg, opt[RROW:RROW + 1, :])
                        nc.scalar.activation(t, opt[0:D, :], AF.Copy)
                    nc.sync.dma_start(r8[i:i + 1, :], rg)
                    onh.append(t)
                nc.vector.reciprocal(r8, r8)
                rall = hpool.tile([NH, BLK], f32r, tag="rall")
                nc.vector.tensor_copy(rall, r8)

                # assemble outT (128,512) tiles from head rows via DMA
                outT = []
                for tt in range(KC):
                    t = opool.tile([128, BLK], f32, tag=f"ot{tt}")
                    outT.append(t)
                for (i, tt, lo, hi, dlo) in PIECES:
                    nc.sync.dma_start(outT[tt][lo:hi, :],
                                      onh[i][dlo:dlo + hi - lo, :])

                # normalize in place: outT_t *= R_t, R_t = S_t^T @ rall
                outn = []
                for tt in range(KC):
                    Rp = psr.tile([128, BLK], f32, tag="psr")
                    nc.tensor.matmul(Rp, S_sel[tt], rall, start=True, stop=True)
                    nc.vector.tensor_mul(outT[tt].bitcast(f32r), outT[tt], Rp)
                    outn.append(outT[tt].bitcast(f32r))

                # y = Wo^T @ outn + bo, straight to the output layout
                for m in range(KC):
                    pt = ppa.tile([128, BLK], f32, tag="ppa")
                    for k in range(KC):
                        nc.tensor.matmul(
                            pt, wo_sb[k][:, 128 * m:128 * (m + 1)], outn[k],
                            start=(k == 0), stop=(k == KC - 1))
                    t = fpool.tile([128, BLK], f32, tag="fin")
                    nc.scalar.activation(t, pt, AF.Identity,
                                         bias=boT[:, m:m + 1])
                    nc.sync.dma_start(
                        out_d[b, 128 * m:128 * (m + 1), c0:c0 + BLK], t)
    nc.compile()
    return nc


def _get_nc():
    if "nc" not in _CACHE:
        _CACHE["nc"] = _build_nc()
    return _CACHE["nc"]


def kernel(**inputs):
    from concourse.bass_utils import run_bass_kernel_spmd

    x = np.asarray(inputs["x"], dtype=np.float32)
    context = np.asarray(inputs["context"], dtype=np.float32)
    wq = np.ascontiguousarray(np.asarray(inputs["Wq"], dtype=np.float32))
    wk = np.ascontiguousarray(np.asarray(inputs["Wk"], dtype=np.float32))
    wv = np.ascontiguousarray(np.asarray(inputs["Wv"], dtype=np.float32))
    wo = np.ascontiguousarray(np.asarray(inputs["Wo"], dtype=np.float32))
    bq = np.ascontiguousarray(np.asarray(inputs["bq"], dtype=np.float32))
    bk = np.ascontiguousarray(np.asarray(inputs["bk"], dtype=np.float32))
    bv = np.ascontiguousarray(np.asarray(inputs["bv"], dtype=np.float32))
    bo = np.ascontiguousarray(np.asarray(inputs["bo"], dtype=np.float32))

    xs = np.ascontiguousarray(x.reshape(B, C, NTOK))
    ctxs = np.ascontiguousarray(context)

    nc = _get_nc()
    in_maps = []
    for c in range(N_CORES):
        sl = slice(BPC * c, BPC * (c + 1))
        in_maps.append({
            "x": np.ascontiguousarray(xs[sl]),
            "ctxt": np.ascontiguousarray(ctxs[sl]),
            "wq": wq, "wk": wk, "wv": wv, "wo": wo,
            "bq": bq, "bk": bk, "bv": bv, "bo": bo,
        })
    res = run_bass_kernel_spmd(nc, in_maps, list(range(N_CORES))).results
    out = np.concatenate([res[c]["out"] for c in range(N_CORES)], axis=0)
    return np.ascontiguousarray(out.reshape(B, C, HH, WW))

